# revision 7
# baseline (speedup 1.0000x reference)
"""COIL-style retrieval scoring kernel for Trainium2 (8 NeuronCores, SPMD).

Problem: nn_BertForSemanticEmbedding_16973710754315
  out[q, n] = sum_{i>=1} mask[q,i] * max_j( where(qid[q,i]==did[n,j], qry[q,i]·doc[n,j], 0) )

Algorithm (per core, docs sharded 16 docs/core, queries replicated):
  * Fold the exact-match mask INTO the matmul: augment each 32-dim token
    vector with a 96-dim signature code C[id] with entries +-4. Then the
    K=128 matmul computes  S' = S + code(qid)·code(did), where a matching
    id contributes exactly BIG = 96*16 = 1536 and a mismatching id at most
    736 (verified max Gram off-diagonal). Since |S| < ~50, thresholding
    relu(x - 1536) after a per-doc max over doc tokens recovers
    relu(max over matching j of S) exactly -- which equals the reference's
    where(...).max(axis) whenever at least one non-match exists per
    (token, doc) (always true here; verified on the data).
  * Per query q (128 tokens on partitions x 2048 doc-token columns in PSUM):
      - TensorE: 4 matmuls N=512 (bf16, K=128).
      - VectorE: direct segmented reduce_max from PSUM for docs 0..4.
      - ScalarE: relu(x-1536) extraction to bf16 SBUF for docs 5..15.
  * VectorE: batched binary max-tree over the bf16 extraction (2x mode).
  * Mask multiply (bf16) + final masked sum over query tokens via a
    ones-vector matmul on TensorE; DMA the [1, 256] result out.
"""

import sys
import numpy as np

for _p in ("/opt/trn_rl_repo",):
    if _p not in sys.path:
        sys.path.insert(0, _p)

import ml_dtypes

BF16 = ml_dtypes.bfloat16

NQ, LQ = 16, 128
ND, LD = 128, 128
D = 32
VOCAB = 1000
R = 96                 # signature code dims
CVAL = 4.0             # code entry magnitude (exact in bf16)
BIG = R * CVAL * CVAL  # 1536.0 == exact self-dot of every code row
KAUG = D + R           # 128 = full PE contraction dim
NCORES = 8
DSHARD = ND // NCORES  # 16 docs per core
NDTOK = DSHARD * LD    # 2048 doc tokens per core
NQTOK = NQ * LQ        # 2048 query tokens
F_DVE = 4              # docs reduced directly on VectorE from PSUM
F_ACT = DSHARD - F_DVE # docs extracted by ScalarE + max-tree
QBATCH = 4             # queries per tree batch (tree interleaved per batch)

_CODE = None


def _code():
    """[VOCAB, R] code matrix, entries +-CVAL. Deterministic; margin was
    verified offline: max off-diagonal Gram entry 736 << BIG - max|S|."""
    global _CODE
    if _CODE is None:
        rng = np.random.RandomState(12345)
        _CODE = np.where(rng.rand(VOCAB, R) < 0.5, -CVAL, CVAL).astype(np.float32)
    return _CODE


def _build_program():
    from concourse import bacc, tile, mybir

    bf = mybir.dt.bfloat16
    f32 = mybir.dt.float32

    nc = bacc.Bacc("TRN2", target_bir_lowering=False, debug=False,
                   num_devices=NCORES)
    # register the relu-threshold bias constant (activation() requires a
    # pre-registered const AP for float biases)
    _bias_t = nc.alloc_sbuf_tensor("const-float32--1536", [128, 1],
                                   mybir.dt.float32)
    nc.gpsimd.memset(_bias_t.ap(), -float(BIG))
    nc.const_aps.aps[(mybir.dt.float32, -float(BIG))] = _bias_t.ap()

    qT_d = nc.declare_dram_parameter("qT", [KAUG, NQTOK], bf, isOutput=False)
    dT_d = nc.declare_dram_parameter("dT", [KAUG, NDTOK], bf, isOutput=False)
    wf_d = nc.declare_dram_parameter("wf", [LQ, NQ, DSHARD], bf, isOutput=False)
    on_d = nc.declare_dram_parameter("ones", [KAUG, 1], bf, isOutput=False)
    out_d = nc.declare_dram_parameter("out", [1, NQ * DSHARD], f32, isOutput=True)

    NCHUNK = 4
    CW = NDTOK // NCHUNK  # 512 columns per matmul

    NB = NQ // QBATCH  # tree batches

    with tile.TileContext(nc) as tc:
        with (
            tc.tile_pool(name="io", bufs=1) as io,
            tc.tile_pool(name="ebuf", bufs=2) as ebuf,
            tc.tile_pool(name="small", bufs=1) as small,
            tc.tile_pool(name="ps", bufs=2, space="PSUM") as ps,
        ):
            # q0's first matmul needs only QT chunk 0 + DT chunk 0: issue
            # those first so compute starts while the rest streams in.
            QTc = []
            qchunk = NQTOK // NCHUNK
            for c in range(NCHUNK):
                t = io.tile([KAUG, qchunk], bf, tag=f"qt{c}")
                QTc.append(t)
            DTc = []
            for c in range(NCHUNK):
                t = io.tile([KAUG, CW], bf, tag=f"dt{c}")
                DTc.append(t)
            nc.sync.dma_start(QTc[0][:], qT_d[:, 0:qchunk])
            nc.sync.dma_start(DTc[0][:], dT_d[:, 0:CW])
            for c in range(1, NCHUNK):
                nc.sync.dma_start(DTc[c][:], dT_d[:, c * CW:(c + 1) * CW])
            for c in range(1, NCHUNK):
                nc.sync.dma_start(QTc[c][:], qT_d[:, c * qchunk:(c + 1) * qchunk])
            WF = small.tile([LQ, NQ, DSHARD], bf, tag="wf")
            nc.sync.dma_start(WF[:], wf_d[:])
            ONES = small.tile([KAUG, 1], bf, tag="ones")
            nc.sync.dma_start(ONES[:], on_d[:])

            Mdve = small.tile([LQ, NQ, F_DVE], f32, tag="mdve")
            Rall = small.tile([LQ, NQ, DSHARD], bf, tag="rall")
            OUTS = small.tile([1, NQ * DSHARD], f32, tag="outs")

            docs_per_chunk = CW // LD  # 4
            q_per_chunk = qchunk // LQ  # 4

            for b in range(NB):
                E4 = ebuf.tile([LQ, QBATCH, F_ACT, LD], bf, tag="e")
                for qq in range(QBATCH):
                    q = b * QBATCH + qq
                    # PSUM tile viewed as [128 qtok, 16 docs, 128 doctok]
                    psq = ps.tile([LQ, DSHARD, LD], f32, tag="ps")
                    lhs = QTc[q // q_per_chunk][
                        :, (q % q_per_chunk) * LQ:((q % q_per_chunk) + 1) * LQ]
                    for c in range(NCHUNK):
                        nc.tensor.matmul(
                            psq[:, c * docs_per_chunk:(c + 1) * docs_per_chunk, :],
                            lhs,
                            DTc[c][:],
                            start=True, stop=True,
                        )
                    # VectorE: direct segmented max for docs 0..F_DVE-1
                    nc.vector.reduce_max(
                        Mdve[:, q, :], psq[:, 0:F_DVE, :],
                        axis=mybir.AxisListType.X,
                    )
                    # ScalarE: relu(x - BIG) for docs F_DVE..15, bf16 out
                    nc.scalar.activation(
                        E4[:, qq, :, :], psq[:, F_DVE:DSHARD, :],
                        mybir.ActivationFunctionType.Relu,
                        bias=-float(BIG),
                    )

                # binary max-tree over this batch: [128, 4, F_ACT, 128]
                cur = E4[:]
                width = LD
                lev = 0
                while width > 2:
                    half = width // 2
                    t = ebuf.tile([LQ, QBATCH, F_ACT, half], bf, tag=f"tr{lev}")
                    nc.vector.tensor_max(t[:], cur[:, :, :, 0:half],
                                         cur[:, :, :, half:width])
                    cur = t[:]
                    width = half
                    lev += 1
                qlo, qhi = b * QBATCH, (b + 1) * QBATCH
                nc.vector.tensor_max(Rall[:, qlo:qhi, F_DVE:DSHARD],
                                     cur[:, :, :, 0], cur[:, :, :, 1])
                # DVE-direct docs: relu(x - BIG) into Rall[:, qlo:qhi, :F_DVE]
                nc.vector.tensor_scalar(
                    Rall[:, qlo:qhi, 0:F_DVE], Mdve[:, qlo:qhi, :],
                    float(BIG), 0.0,
                    op0=mybir.AluOpType.subtract, op1=mybir.AluOpType.max,
                )
                # mask multiply + masked sum over query tokens (ones-matmul)
                Rm = ebuf.tile([LQ, QBATCH, DSHARD], bf, tag="rm")
                nc.vector.tensor_mul(Rm[:], Rall[:, qlo:qhi, :],
                                     WF[:, qlo:qhi, :])
                pso_b = ps.tile([1, QBATCH * DSHARD], f32, tag="ps")
                nc.tensor.matmul(pso_b[:], ONES[:], Rm[:],
                                 start=True, stop=True)
                nc.vector.tensor_copy(
                    OUTS[:, qlo * DSHARD:qhi * DSHARD], pso_b[:])

            nc.sync.dma_start(out_d[:], OUTS[:])

    nc.compile()
    return nc


_NC = None


def _get_nc():
    global _NC
    if _NC is None:
        _NC = _build_program()
    return _NC


def _install_ntff_shim():
    """Under axon the NTFF profile hook module may be missing; install it so
    trace=True returns exec_time_ns. Harmless no-op if already present."""
    import types
    try:
        import antenv.axon_hooks  # noqa: F401
        return
    except ImportError:
        pass
    try:
        from trn_agent_boot.trn_boot import _ntff_profile_via_ctypes
        hook = _ntff_profile_via_ctypes("/opt/axon/libaxon_pjrt.so")
        mod = types.ModuleType("antenv.axon_hooks")
        mod.get_axon_ntff_profile_hook = lambda: hook
        mod.set_axon_ntff_profile_hook = lambda h: None
        sys.modules["antenv.axon_hooks"] = mod
    except Exception:
        pass


def _prep_in_maps(doc_reps, qry_reps, qry_attention_mask, doc_input_ids,
                  qry_input_ids):
    C = _code()
    qry_reps = np.asarray(qry_reps, dtype=np.float32)
    doc_reps = np.asarray(doc_reps, dtype=np.float32)
    mask = np.asarray(qry_attention_mask, dtype=np.float32)
    qids = np.asarray(qry_input_ids).astype(np.int64).reshape(-1)
    dids = np.asarray(doc_input_ids).astype(np.int64).reshape(-1)

    Qaug = np.concatenate(
        [qry_reps.reshape(NQTOK, D), C[qids]], axis=1).astype(BF16)
    Daug = np.concatenate(
        [doc_reps.reshape(ND * LD, D), C[dids]], axis=1).astype(BF16)
    qT = np.ascontiguousarray(Qaug.T)  # [128, 2048]

    W = mask.copy()
    W[:, 0] = 0.0  # skip [CLS]
    # wf[i, q, n] = W[q, i]
    wf = np.ascontiguousarray(
        np.broadcast_to(W.T[:, :, None], (LQ, NQ, DSHARD))).astype(BF16)
    ones = np.ones((KAUG, 1), dtype=BF16)

    in_maps = []
    for core in range(NCORES):
        shard = Daug[core * NDTOK:(core + 1) * NDTOK]
        dT = np.ascontiguousarray(shard.T)  # [128, 2048]
        in_maps.append({"qT": qT, "dT": dT, "wf": wf, "ones": ones})
    return in_maps


def _run(in_maps, trace=False):
    from concourse.bass_utils import run_bass_kernel_spmd
    if trace:
        _install_ntff_shim()
    nc = _get_nc()
    res = run_bass_kernel_spmd(nc, in_maps, core_ids=list(range(NCORES)),
                               trace=trace)
    out = np.zeros((NQ, ND), dtype=np.float32)
    for core in range(NCORES):
        out[:, core * DSHARD:(core + 1) * DSHARD] = \
            res.results[core]["out"].reshape(NQ, DSHARD)
    return out, res


def kernel(doc_reps, qry_reps, qry_attention_mask, doc_input_ids,
           qry_input_ids):
    in_maps = _prep_in_maps(doc_reps, qry_reps, qry_attention_mask,
                            doc_input_ids, qry_input_ids)
    out, _ = _run(in_maps, trace=False)
    return out


def kernel_traced(doc_reps, qry_reps, qry_attention_mask, doc_input_ids,
                  qry_input_ids):
    """Returns (output, exec_time_ns) using the NTFF profiling path."""
    in_maps = _prep_in_maps(doc_reps, qry_reps, qry_attention_mask,
                            doc_input_ids, qry_input_ids)
    out, res = _run(in_maps, trace=True)
    return out, res.exec_time_ns


# revision 8
# speedup vs baseline: 1.3910x; 1.3910x over previous
"""COIL-style retrieval scoring kernel for Trainium2 (8 NeuronCores, SPMD).

Problem: nn_BertForSemanticEmbedding_16973710754315
  out[q, n] = sum_{i>=1} mask[q,i] * max_j( where(qid[q,i]==did[n,j], qry[q,i]·doc[n,j], 0) )

Algorithm (per core, docs sharded 16 docs/core, queries replicated):
  * Fold the exact-match mask INTO the matmul: augment each 32-dim token
    vector with a 96-dim signature code C[id] with entries +-4. Then the
    K=128 matmul computes  S' = S + code(qid)·code(did), where a matching
    id contributes exactly BIG = 96*16 = 1536 and a mismatching id at most
    736 (verified max Gram off-diagonal). Since |S| < ~50, thresholding
    relu(x - 1536) after a per-doc max over doc tokens recovers
    relu(max over matching j of S) exactly -- which equals the reference's
    where(...).max(axis) whenever at least one non-match exists per
    (token, doc) (always true here; verified on the data).
  * Per query q (128 tokens on partitions x 2048 doc-token columns in PSUM):
      - TensorE: 4 matmuls N=512 (bf16, K=128).
      - VectorE: direct segmented reduce_max from PSUM for docs 0..4.
      - ScalarE: relu(x-1536) extraction to bf16 SBUF for docs 5..15.
  * VectorE: batched binary max-tree over the bf16 extraction (2x mode).
  * Mask multiply (bf16) + final masked sum over query tokens via a
    ones-vector matmul on TensorE; DMA the [1, 256] result out.
"""

import sys
import numpy as np

for _p in ("/opt/trn_rl_repo",):
    if _p not in sys.path:
        sys.path.insert(0, _p)

import ml_dtypes

BF16 = ml_dtypes.bfloat16

NQ, LQ = 16, 128
ND, LD = 128, 128
D = 32
VOCAB = 1000
R = 96                 # signature code dims
CVAL = 4.0             # code entry magnitude (exact in bf16)
BIG = R * CVAL * CVAL  # 1536.0 == exact self-dot of every code row
KAUG = D + R           # 128 = full PE contraction dim
NCORES = 8
DSHARD = ND // NCORES  # 16 docs per core
NDTOK = DSHARD * LD    # 2048 doc tokens per core
NQTOK = NQ * LQ        # 2048 query tokens
F_DVE = 4              # docs reduced directly on VectorE from PSUM
F_ACT = DSHARD - F_DVE # docs extracted by ScalarE + max-tree
QBATCH = 4             # queries per tree batch (tree interleaved per batch)

_CODE = None


def _code():
    """[VOCAB, R] code matrix, entries +-CVAL. Deterministic; margin was
    verified offline: max off-diagonal Gram entry 736 << BIG - max|S|."""
    global _CODE
    if _CODE is None:
        rng = np.random.RandomState(12345)
        _CODE = np.where(rng.rand(VOCAB, R) < 0.5, -CVAL, CVAL).astype(np.float32)
    return _CODE


def _build_program():
    from concourse import bacc, tile, mybir

    bf = mybir.dt.bfloat16
    f32 = mybir.dt.float32

    nc = bacc.Bacc("TRN2", target_bir_lowering=False, debug=False,
                   num_devices=NCORES)
    # register the relu-threshold bias constant (activation() requires a
    # pre-registered const AP for float biases)
    _bias_t = nc.alloc_sbuf_tensor("const-float32--1536", [128, 1],
                                   mybir.dt.float32)
    nc.gpsimd.memset(_bias_t.ap(), -float(BIG))
    nc.const_aps.aps[(mybir.dt.float32, -float(BIG))] = _bias_t.ap()

    qT_d = nc.declare_dram_parameter("qT", [KAUG, NQTOK], bf, isOutput=False)
    dT_d = nc.declare_dram_parameter("dT", [KAUG, NDTOK], bf, isOutput=False)
    wf_d = nc.declare_dram_parameter("wf", [LQ, NQ, DSHARD], bf, isOutput=False)
    on_d = nc.declare_dram_parameter("ones", [KAUG, 1], bf, isOutput=False)
    out_d = nc.declare_dram_parameter("out", [1, NQ * DSHARD], f32, isOutput=True)

    NCHUNK = 4
    CW = NDTOK // NCHUNK  # 512 columns per matmul

    NB = NQ // QBATCH  # tree batches

    with tile.TileContext(nc) as tc:
        with (
            tc.tile_pool(name="io", bufs=1) as io,
            tc.tile_pool(name="ebuf", bufs=2) as ebuf,
            tc.tile_pool(name="small", bufs=1) as small,
            tc.tile_pool(name="ps", bufs=2, space="PSUM") as ps,
        ):
            # q0's first matmul needs only QT chunk 0 + DT chunk 0: issue
            # those first so compute starts while the rest streams in.
            QTc = []
            qchunk = NQTOK // NCHUNK
            for c in range(NCHUNK):
                t = io.tile([KAUG, qchunk], bf, tag=f"qt{c}")
                QTc.append(t)
            DTc = []
            for c in range(NCHUNK):
                t = io.tile([KAUG, CW], bf, tag=f"dt{c}")
                DTc.append(t)
            nc.sync.dma_start(QTc[0][:], qT_d[:, 0:qchunk])
            nc.sync.dma_start(DTc[0][:], dT_d[:, 0:CW])
            for c in range(1, NCHUNK):
                nc.sync.dma_start(DTc[c][:], dT_d[:, c * CW:(c + 1) * CW])
            for c in range(1, NCHUNK):
                nc.sync.dma_start(QTc[c][:], qT_d[:, c * qchunk:(c + 1) * qchunk])
            WF = small.tile([LQ, NQ, DSHARD], bf, tag="wf")
            nc.sync.dma_start(WF[:], wf_d[:])
            ONES = small.tile([KAUG, 1], bf, tag="ones")
            nc.sync.dma_start(ONES[:], on_d[:])

            Mdve = small.tile([LQ, NQ, F_DVE], f32, tag="mdve")
            Rall = small.tile([LQ, NQ, DSHARD], bf, tag="rall")
            OUTS = small.tile([1, NQ * DSHARD], f32, tag="outs")

            docs_per_chunk = CW // LD  # 4
            q_per_chunk = qchunk // LQ  # 4

            for b in range(NB):
                E4 = ebuf.tile([LQ, QBATCH, F_ACT, LD], bf, tag="e")
                for qq in range(QBATCH):
                    q = b * QBATCH + qq
                    # PSUM tile viewed as [128 qtok, 16 docs, 128 doctok]
                    psq = ps.tile([LQ, DSHARD, LD], f32, tag="ps")
                    lhs = QTc[q // q_per_chunk][
                        :, (q % q_per_chunk) * LQ:((q % q_per_chunk) + 1) * LQ]
                    for c in range(NCHUNK):
                        nc.tensor.matmul(
                            psq[:, c * docs_per_chunk:(c + 1) * docs_per_chunk, :],
                            lhs,
                            DTc[c][:],
                            start=True, stop=True,
                        )
                    # VectorE: direct segmented max for docs 0..F_DVE-1
                    nc.vector.reduce_max(
                        Mdve[:, q, :], psq[:, 0:F_DVE, :],
                        axis=mybir.AxisListType.X,
                    )
                    # ScalarE: relu(x - BIG) for docs F_DVE..15, bf16 out
                    nc.scalar.activation(
                        E4[:, qq, :, :], psq[:, F_DVE:DSHARD, :],
                        mybir.ActivationFunctionType.Relu,
                        bias=-float(BIG),
                    )

                # binary max-tree over this batch: [128, 4, F_ACT, 128]
                cur = E4[:]
                width = LD
                lev = 0
                while width > 2:
                    half = width // 2
                    t = ebuf.tile([LQ, QBATCH, F_ACT, half], bf, tag=f"tr{lev}")
                    nc.vector.tensor_max(t[:], cur[:, :, :, 0:half],
                                         cur[:, :, :, half:width])
                    cur = t[:]
                    width = half
                    lev += 1
                qlo, qhi = b * QBATCH, (b + 1) * QBATCH
                nc.vector.tensor_max(Rall[:, qlo:qhi, F_DVE:DSHARD],
                                     cur[:, :, :, 0], cur[:, :, :, 1])
                # DVE-direct docs: relu(x - BIG) into Rall[:, qlo:qhi, :F_DVE]
                nc.vector.tensor_scalar(
                    Rall[:, qlo:qhi, 0:F_DVE], Mdve[:, qlo:qhi, :],
                    float(BIG), 0.0,
                    op0=mybir.AluOpType.subtract, op1=mybir.AluOpType.max,
                )

            # end phase: mask multiply, masked query-token sum (ones-matmul),
            # copy out.  Kept out of the main loop so the PE instruction
            # stream stays dense (in-order engine queues).
            Rm = small.tile([LQ, NQ, DSHARD], bf, tag="rm")
            nc.vector.tensor_mul(Rm[:], Rall[:], WF[:])
            pso = ps.tile([1, NQ * DSHARD], f32, tag="ps")
            nc.tensor.matmul(pso[:], ONES[:], Rm[:], start=True, stop=True)
            nc.vector.tensor_copy(OUTS[:], pso[:])
            nc.sync.dma_start(out_d[:], OUTS[:])

    nc.compile()
    return nc


_NC = None


def _get_nc():
    global _NC
    if _NC is None:
        _NC = _build_program()
    return _NC


def _install_ntff_shim():
    """Under axon the NTFF profile hook module may be missing; install it so
    trace=True returns exec_time_ns. Harmless no-op if already present."""
    import types
    try:
        import antenv.axon_hooks  # noqa: F401
        return
    except ImportError:
        pass
    try:
        from trn_agent_boot.trn_boot import _ntff_profile_via_ctypes
        hook = _ntff_profile_via_ctypes("/opt/axon/libaxon_pjrt.so")
        mod = types.ModuleType("antenv.axon_hooks")
        mod.get_axon_ntff_profile_hook = lambda: hook
        mod.set_axon_ntff_profile_hook = lambda h: None
        sys.modules["antenv.axon_hooks"] = mod
    except Exception:
        pass


def _prep_in_maps(doc_reps, qry_reps, qry_attention_mask, doc_input_ids,
                  qry_input_ids):
    C = _code()
    qry_reps = np.asarray(qry_reps, dtype=np.float32)
    doc_reps = np.asarray(doc_reps, dtype=np.float32)
    mask = np.asarray(qry_attention_mask, dtype=np.float32)
    qids = np.asarray(qry_input_ids).astype(np.int64).reshape(-1)
    dids = np.asarray(doc_input_ids).astype(np.int64).reshape(-1)

    Qaug = np.concatenate(
        [qry_reps.reshape(NQTOK, D), C[qids]], axis=1).astype(BF16)
    Daug = np.concatenate(
        [doc_reps.reshape(ND * LD, D), C[dids]], axis=1).astype(BF16)
    qT = np.ascontiguousarray(Qaug.T)  # [128, 2048]

    W = mask.copy()
    W[:, 0] = 0.0  # skip [CLS]
    # wf[i, q, n] = W[q, i]
    wf = np.ascontiguousarray(
        np.broadcast_to(W.T[:, :, None], (LQ, NQ, DSHARD))).astype(BF16)
    ones = np.ones((KAUG, 1), dtype=BF16)

    in_maps = []
    for core in range(NCORES):
        shard = Daug[core * NDTOK:(core + 1) * NDTOK]
        dT = np.ascontiguousarray(shard.T)  # [128, 2048]
        in_maps.append({"qT": qT, "dT": dT, "wf": wf, "ones": ones})
    return in_maps


def _run(in_maps, trace=False):
    from concourse.bass_utils import run_bass_kernel_spmd
    if trace:
        _install_ntff_shim()
    nc = _get_nc()
    res = run_bass_kernel_spmd(nc, in_maps, core_ids=list(range(NCORES)),
                               trace=trace)
    out = np.zeros((NQ, ND), dtype=np.float32)
    for core in range(NCORES):
        out[:, core * DSHARD:(core + 1) * DSHARD] = \
            res.results[core]["out"].reshape(NQ, DSHARD)
    return out, res


def kernel(doc_reps, qry_reps, qry_attention_mask, doc_input_ids,
           qry_input_ids):
    in_maps = _prep_in_maps(doc_reps, qry_reps, qry_attention_mask,
                            doc_input_ids, qry_input_ids)
    out, _ = _run(in_maps, trace=False)
    return out


def kernel_traced(doc_reps, qry_reps, qry_attention_mask, doc_input_ids,
                  qry_input_ids):
    """Returns (output, exec_time_ns) using the NTFF profiling path."""
    in_maps = _prep_in_maps(doc_reps, qry_reps, qry_attention_mask,
                            doc_input_ids, qry_input_ids)
    out, res = _run(in_maps, trace=True)
    return out, res.exec_time_ns
